# revision 24
# baseline (speedup 1.0000x reference)
"""Trainium2 Bass kernel for CoAttention (fp16 data path).

Math (per batch b):
    q_sum = sum_q(sentence) @ Wq.T + Lq*bq          [D]
    w     = q_sum @ Wk                              [D]   (bk dropped: softmax shift-invariant)
    s_k   = comment[k] . w                          [Lk]
    p     = exp(s - max s);  l = sum p
    ctx   = (p @ comment) / l                       [D]
    out   = ctx @ Wv.T + bv                         [D]

Sharding: data-parallel over batch, 4 batches per core, weights replicated.

The kernel is DMA-bound: every byte of sentence/comment is read exactly once
and FLOPs are negligible, so all bulk tensors are converted to fp16 on the
host (rel err ~5e-4, verified against the f32 reference) and pre-permuted so
every DMA is a large contiguous transfer. Per-core traffic: 12.6 MiB comment
+ 3.1 MiB sentence + 3.4 MiB weights ~= 19 MiB -> ~53 us at 358 GB/s.

Engine plan per core (4 batches, whole comment shard resident in SBUF):
  - scores: fused mul+reduce (scalar_tensor_tensor) on DVE, fp16 in / f32 accum
  - softmax: DVE row-max, PE-transpose cross-partition max, ACT exp w/ accum
  - sentence sums, w chain, ctx accumulation, projections: PE fp16
  - bulk DMA on the SP HWDGE ring; small latency-critical DMAs on the ACT ring
"""

import numpy as np

B, LQ, LK, D = 32, 512, 2048, 768
NCORES = 8
BPC = B // NCORES      # 4 batches per core
KT = LK // 128         # 16 k-tiles per batch
QT = LQ // 128         # 4 q-tiles per batch
DC = D // 128          # 6 d-chunks
KPS = 4                # k-tiles per DMA slab
NSLAB = KT // KPS      # 4 slabs per batch

_cache = {}


def _split_multi_waits(nc):
    """This walrus build allows only ONE sync-wait command per instruction.
    Tile emits several when an instruction depends on multiple procs. Hoist
    the extras onto same-engine NoOps inserted immediately before (the engine
    queue is FIFO, so the waits execute in order — semantically identical)."""
    import bass_rust
    from concourse import mybir

    n_split = 0
    for f in nc.m.functions:
        for bb in f.blocks:
            out = []
            for inst in bb.instructions:
                si = inst.sync_info
                waits = list(si.on_wait or []) if si else []
                if len(waits) > 1:
                    for i, w in enumerate(waits[:-1]):
                        nop = mybir.InstNoOp(name=f"{inst.name}-ws{i}")
                        nop.engine = inst.engine
                        nop.bass_nofuse = True
                        nop.sync_info = bass_rust.SyncInfo(
                            on_wait=[w], on_update=[]
                        )
                        out.append(nop)
                        n_split += 1
                    si.on_wait = waits[-1:]
                out.append(inst)
            bb.instructions[:] = out
    return n_split


def build_program(split_waits=True, reps=1):
    import contextlib

    import concourse.bass as bass
    import concourse.tile as tile
    from concourse import masks, mybir

    f16 = mybir.dt.float16
    f32 = mybir.dt.float32
    Alu = mybir.AluOpType
    Act = mybir.ActivationFunctionType
    Axis = mybir.AxisListType

    nc = bass.Bass()
    # host-pre-permuted layouts: partition dim second, fully contiguous DMAs
    sent = nc.declare_dram_parameter("sent", [BPC, 128, QT, D], f16, isOutput=False)
    comm = nc.declare_dram_parameter("comm", [BPC, 128, KT, D], f16, isOutput=False)
    wqt = nc.declare_dram_parameter("wqt", [128, DC, D], f16, isOutput=False)
    wk = nc.declare_dram_parameter("wk", [128, DC, D], f16, isOutput=False)
    wvt = nc.declare_dram_parameter("wvt", [128, DC, D], f16, isOutput=False)
    bq = nc.declare_dram_parameter("bq", [D], f16, isOutput=False)
    bv = nc.declare_dram_parameter("bv", [D], f16, isOutput=False)
    out = nc.declare_dram_parameter("out", [BPC, D], f32, isOutput=True)

    with tile.TileContext(nc) as tc:
      rep_loop = tc.For_i(0, reps, 1) if reps > 1 else contextlib.nullcontext()
      with rep_loop:
        with (
            tc.tile_pool(name="consts", bufs=1) as consts,
            tc.tile_pool(name="rows", bufs=1) as rows,
            tc.tile_pool(name="smalls", bufs=2) as smalls,
            tc.tile_pool(name="wp", bufs=1) as wp,
            tc.tile_pool(name="sentp", bufs=BPC) as sentp,
            tc.tile_pool(name="commp", bufs=BPC * NSLAB) as commp,
            tc.tile_pool(name="wbp", bufs=BPC) as wbp,
            tc.tile_pool(name="scr", bufs=1) as scr,
            tc.tile_pool(name="dramp", bufs=1, space="DRAM") as dramp,
            tc.tile_pool(name="ps", bufs=1, space="PSUM") as ps,
        ):
            # ---------------- constants (no DMA) ----------------
            ident = consts.tile([128, 128], f32)
            masks.make_identity(nc, ident[:])
            ones_col16 = consts.tile([128, 1], f16)
            nc.vector.memset(ones_col16[:], 1.0)
            ones_col32 = consts.tile([128, 1], f32)
            nc.vector.memset(ones_col32[:], 1.0)
            ones_row16 = consts.tile([1, 128], f16)
            nc.vector.memset(ones_row16[:], 1.0)
            ones_row32 = consts.tile([1, 128], f32)
            nc.vector.memset(ones_row32[:], 1.0)
            lq_row = consts.tile([1, 128], f16)
            nc.vector.memset(lq_row[:], float(LQ))
            dummy = consts.tile([1, 1], f32)
            nc.vector.memset(dummy[:], 0.0)
            nc.scalar.activation(dummy[:], dummy[:], Act.Exp)

            # ---------------- DMA issue order on the SP ring -------------
            # Wq/Wk (phase-0 critical) -> sentence -> comment -> Wv (end)
            wqt_sb = wp.tile([128, DC, D], f16, tag="wqt")
            nc.sync.dma_start(out=wqt_sb[:], in_=wqt[:])
            wk_sb = wp.tile([128, DC, D], f16, tag="wk")
            nc.sync.dma_start(out=wk_sb[:], in_=wk[:])

            sent_tiles = []
            for b in range(BPC):
                t = sentp.tile([128, QT, D], f16, tag="sent")
                nc.sync.dma_start(out=t[:], in_=sent[b])
                sent_tiles.append(t)

            slabs = {}
            for b in range(BPC):
                for s in range(NSLAB):
                    t = commp.tile([128, KPS, D], f16, tag="slab")
                    nc.sync.dma_start(
                        out=t[:], in_=comm[b, :, s * KPS : (s + 1) * KPS, :]
                    )
                    slabs[(b, s)] = t

            wvt_sb = wp.tile([128, DC, D], f16, tag="wvt")
            nc.sync.dma_start(out=wvt_sb[:], in_=wvt[:])

            # small loads on the ACT ring (bypass the bulk FIFO)
            bq_row = rows.tile([1, D], f16, tag="bq")
            nc.scalar.dma_start(out=bq_row[:], in_=bq[None, :])
            bv_row = rows.tile([1, D], f16, tag="bv")
            nc.scalar.dma_start(out=bv_row[:], in_=bv[None, :])

            # ---------------- phase 0: s_sum (column form), q_sum, w -----
            # ssT[:, c, b] = sum over q of sentence[b, q, c*128:(c+1)*128]
            ssT = smalls.tile([128, DC, BPC], f16, tag="ssT")
            for b in range(BPC):
                for c in range(DC):
                    pc = ps.tile([128, 1], f32, tag="C", bufs=3)
                    for t in range(QT):
                        nc.tensor.matmul(
                            pc[:],
                            sent_tiles[b][:, t, c * 128 : (c + 1) * 128],
                            ones_col16[:],
                            start=(t == 0), stop=(t == QT - 1),
                        )
                    nc.scalar.copy(ssT[:, c, b : b + 1], pc[:])

            # q_sumT chunks [128, DC, BPC]: Wq @ ssT + Lq*bq
            q_sumT = smalls.tile([128, DC, BPC], f16, tag="qsT")
            for m in range(DC):
                pq = ps.tile([128, BPC], f32, tag="C", bufs=3)
                for c in range(DC):
                    nc.tensor.matmul(
                        pq[:], wqt_sb[:, c, m * 128 : (m + 1) * 128],
                        ssT[:, c, :], start=(c == 0), stop=False,
                    )
                nc.tensor.matmul(
                    pq[:], bq_row[0:1, m * 128 : (m + 1) * 128],
                    lq_row[0:1, 0:BPC], start=False, stop=True,
                )
                nc.scalar.copy(q_sumT[:, m, :], pq[:])

            # wT chunks [128, DC, BPC]: Wk.T @ q_sumT
            wT = smalls.tile([128, DC, BPC], f32, tag="wT")
            for m in range(DC):
                pw = ps.tile([128, BPC], f32, tag="C", bufs=3)
                for c in range(DC):
                    nc.tensor.matmul(
                        pw[:], wk_sb[:, c, m * 128 : (m + 1) * 128],
                        q_sumT[:, c, :], start=(c == 0), stop=(c == DC - 1),
                    )
                nc.scalar.copy(wT[:, m, :], pw[:])

            # w rows [BPC, D] fp16 -> DRAM bounce for partition-broadcast
            w_rows = rows.tile([BPC, D], f16, tag="wrows")
            for m in range(DC):
                pr = ps.tile([128, 128], f32, tag="C", bufs=3)
                nc.tensor.transpose(pr[0:BPC, 0:128], wT[:, m, :], ident[:, 0:128])
                nc.scalar.copy(w_rows[:, m * 128 : (m + 1) * 128], pr[0:BPC, 0:128])
            w_dram = dramp.tile([BPC, D], f16)
            nc.scalar.dma_start(out=w_dram[:], in_=w_rows[:])

            # ---------------- main loop over batches ----------------
            import concourse.bass as bass_mod

            s_cols = smalls.tile([128, BPC, KT], f32, tag="scols")
            p_cols = smalls.tile([128, BPC, KT], f16, tag="pcols")
            l_row = rows.tile([1, BPC], f32, tag="lrow")
            ctx_flat = rows.tile([1, BPC, D], f32, tag="ctxflat")
            ctxT = smalls.tile([128, DC, BPC], f16, tag="ctxT")
            # rotated scratches: a single scratch WAW-serializes consecutive
            # DVE score ops behind each write-ack (~0.5us/op measured)
            scratches = [
                scr.tile([128, D], f16, tag=f"ttr{i}", name=f"ttr{i}")
                for i in range(6)
            ]

            for b in range(BPC):
                # broadcast w[b] to all partitions (DRAM source, step-0 AP)
                wb = wbp.tile([128, D], f16, tag="wb")
                src = w_dram[b : b + 1, :]
                src_bcast = bass_mod.AP(
                    tensor=src.tensor, offset=src.offset,
                    ap=[[0, 128]] + list(src.ap[1:]),
                )
                nc.scalar.dma_start(out=wb[:], in_=src_bcast)

                # scores: fused mul+reduce on DVE, one op per k-tile
                for t in range(KT):
                    slab = slabs[(b, t // KPS)]
                    nc.vector.scalar_tensor_tensor(
                        out=scratches[(b * KT + t) % 6][:],
                        in0=slab[:, t % KPS, :],
                        scalar=1.0,
                        in1=wb[:],
                        op0=Alu.mult,
                        op1=Alu.mult,
                        accum_out=s_cols[:, b, t : t + 1],
                    )

                # softmax: row-max on DVE, cross-partition max via PE
                rowmax = smalls.tile([128, 1], f32, tag="rowmax")
                nc.vector.tensor_reduce(
                    out=rowmax[:], in_=s_cols[:, b, :], axis=Axis.X, op=Alu.max
                )
                prm = ps.tile([1, 128], f32, tag="C", bufs=3)
                nc.tensor.transpose(prm[:], rowmax[:], ident[:])
                rm_row = smalls.tile([1, 128], f32, tag="rmrow")
                nc.scalar.copy(rm_row[:], prm[:])
                M_sb = smalls.tile([1, 1], f32, tag="M")
                nc.vector.tensor_reduce(
                    out=M_sb[:], in_=rm_row[:], axis=Axis.X, op=Alu.max
                )
                pnm = ps.tile([128, 1], f32, tag="C", bufs=3)
                nc.tensor.matmul(pnm[:], ones_row32[:], M_sb[:])
                nm = smalls.tile([128, 1], f32, tag="nm")
                nc.scalar.activation(nm[:], pnm[:], Act.Copy, scale=-1.0)
                rowsum = smalls.tile([128, 1], f32, tag="rowsum")
                nc.scalar.activation(
                    p_cols[:, b, :], s_cols[:, b, :], Act.Exp,
                    bias=nm[:], scale=1.0, accum_out=rowsum[:],
                )
                pl = ps.tile([1, 1], f32, tag="C", bufs=3)
                nc.tensor.matmul(pl[:], rowsum[:], ones_col32[:])
                nc.scalar.copy(l_row[0:1, b : b + 1], pl[:])

                # ctx accumulation on PE: ctx = p @ comment
                ca = ps.tile([1, 512], f32, tag="A", bufs=2)
                cb = ps.tile([1, 256], f32, tag="Bk", bufs=2)
                for t in range(KT):
                    slab = slabs[(b, t // KPS)]
                    ch = slab[:, t % KPS, :]
                    pcol = p_cols[:, b, t : t + 1]
                    nc.tensor.matmul(ca[:], pcol, ch[:, 0:512],
                                     start=(t == 0), stop=(t == KT - 1))
                    nc.tensor.matmul(cb[:], pcol, ch[:, 512:768],
                                     start=(t == 0), stop=(t == KT - 1))
                nc.scalar.copy(ctx_flat[0:1, b, 0:512], ca[:])
                nc.scalar.copy(ctx_flat[0:1, b, 512:768], cb[:])

                # ctxT chunks for the final projection, inline per batch
                for c in range(DC):
                    pt = ps.tile([128, BPC], f32, tag="C", bufs=3)
                    nc.tensor.transpose(
                        pt[:, 0:1],
                        ctx_flat[0:1, b, c * 128 : (c + 1) * 128],
                        ident[0:1, 0:1],
                    )
                    nc.scalar.copy(ctxT[:, c, b : b + 1], pt[:, 0:1])

            # ---------------- final projection ----------------
            poa = ps.tile([BPC, 512], f32, tag="A", bufs=2)
            pob = ps.tile([BPC, 256], f32, tag="Bk", bufs=2)
            for c in range(DC):
                nc.tensor.matmul(poa[:], ctxT[:, c, :],
                                 wvt_sb[:, c, 0:512],
                                 start=(c == 0), stop=False)
                nc.tensor.matmul(pob[:], ctxT[:, c, :],
                                 wvt_sb[:, c, 512:768],
                                 start=(c == 0), stop=False)
            nc.tensor.matmul(poa[:], ones_row16[0:1, 0:BPC],
                             bv_row[0:1, 0:512],
                             start=False, stop=True)
            nc.tensor.matmul(pob[:], ones_row16[0:1, 0:BPC],
                             bv_row[0:1, 512:768],
                             start=False, stop=True)

            # 1/l per batch
            pli = ps.tile([BPC, 1], f32, tag="C", bufs=3)
            nc.tensor.transpose(pli[:], l_row[0:1, 0:BPC], ident[0:1, 0:1])
            li = smalls.tile([BPC, 1], f32, tag="li")
            nc.scalar.copy(li[:], pli[:])
            invl = smalls.tile([BPC, 1], f32, tag="invl")
            nc.vector.reciprocal(invl[:], li[:])

            out_sb = rows.tile([BPC, D], f32, tag="outsb")
            nc.vector.tensor_scalar(
                out=out_sb[:, 0:512], in0=poa[:], scalar1=invl[:], scalar2=None,
                op0=Alu.mult,
            )
            nc.vector.tensor_scalar(
                out=out_sb[:, 512:768], in0=pob[:], scalar1=invl[:], scalar2=None,
                op0=Alu.mult,
            )
            nc.scalar.dma_start(out=out[:], in_=out_sb[:])

    if split_waits:
        _split_multi_waits(nc)
    return nc


def _get_program():
    if "nc" not in _cache:
        _cache["nc"] = build_program()
    return _cache["nc"]


def _make_in_maps(sentence_rep, comment_rep, Wq, bq, Wk, bk, Wv, bv):
    del bk  # softmax is shift-invariant: the bk term cancels exactly
    f16 = np.float16

    def to16(x):
        return np.asarray(x, dtype=np.float32).astype(f16)

    # [B, L, D] -> [B, 128, L//128, D]: partition-major, contiguous slab DMAs
    sent = np.ascontiguousarray(
        to16(sentence_rep).reshape(B, QT, 128, D).transpose(0, 2, 1, 3)
    )
    comm = np.ascontiguousarray(
        to16(comment_rep).reshape(B, KT, 128, D).transpose(0, 2, 1, 3)
    )
    # [D, D] -> [128, DC, D] with w[p, c, e] = W[c*128+p, e]
    def wlay(w):
        return np.ascontiguousarray(
            to16(w).reshape(DC, 128, D).transpose(1, 0, 2)
        )

    wqt = wlay(np.asarray(Wq, dtype=np.float32).T)
    wk_ = wlay(np.asarray(Wk, dtype=np.float32))
    wvt = wlay(np.asarray(Wv, dtype=np.float32).T)
    bq_ = to16(bq)
    bv_ = to16(bv)
    in_maps = []
    for c in range(NCORES):
        sl = slice(c * BPC, (c + 1) * BPC)
        in_maps.append({
            "sent": sent[sl], "comm": comm[sl],
            "wqt": wqt, "wk": wk_, "wvt": wvt, "bq": bq_, "bv": bv_,
        })
    return in_maps


def run(inputs, trace=False, **kwargs):
    from concourse.bass_utils import run_bass_kernel_spmd

    nc = _get_program()
    in_maps = _make_in_maps(**inputs)
    res = run_bass_kernel_spmd(
        nc, in_maps, list(range(NCORES)), trace=trace, **kwargs
    )
    out = np.concatenate([res.results[c]["out"] for c in range(NCORES)], axis=0)
    return out.astype(np.float32), res


def kernel(**inputs) -> np.ndarray:
    out, _ = run(inputs)
    return out


# revision 25
# speedup vs baseline: 1.1149x; 1.1149x over previous
"""Trainium2 Bass kernel for CoAttention (fp16 data path).

Math (per batch b):
    q_sum = sum_q(sentence) @ Wq.T + Lq*bq          [D]
    w     = q_sum @ Wk                              [D]   (bk dropped: softmax shift-invariant)
    s_k   = comment[k] . w                          [Lk]
    p     = exp(s - max s);  l = sum p
    ctx   = (p @ comment) / l                       [D]
    out   = ctx @ Wv.T + bv                         [D]

Sharding: data-parallel over batch, 4 batches per core, weights replicated.

The kernel is DMA-bound: every byte of sentence/comment is read exactly once
and FLOPs are negligible, so all bulk tensors are converted to fp16 on the
host (rel err ~5e-4, verified against the f32 reference) and pre-permuted so
every DMA is a large contiguous transfer. Per-core traffic: 12.6 MiB comment
+ 3.1 MiB sentence + 3.4 MiB weights ~= 19 MiB -> ~53 us at 358 GB/s.

Engine plan per core (4 batches, whole comment shard resident in SBUF):
  - scores: fused mul+reduce (scalar_tensor_tensor) on DVE, fp16 in / f32 accum
  - softmax: DVE row-max, PE-transpose cross-partition max, ACT exp w/ accum
  - sentence sums, w chain, ctx accumulation, projections: PE fp16
  - bulk DMA on the SP HWDGE ring; small latency-critical DMAs on the ACT ring
"""

import numpy as np

B, LQ, LK, D = 32, 512, 2048, 768
NCORES = 8
BPC = B // NCORES      # 4 batches per core
KT = LK // 128         # 16 k-tiles per batch
QT = LQ // 128         # 4 q-tiles per batch
DC = D // 128          # 6 d-chunks
KPS = 4                # k-tiles per DMA slab
NSLAB = KT // KPS      # 4 slabs per batch

_cache = {}


def _split_multi_waits(nc):
    """This walrus build allows only ONE sync-wait command per instruction.
    Tile emits several when an instruction depends on multiple procs. Hoist
    the extras onto same-engine NoOps inserted immediately before (the engine
    queue is FIFO, so the waits execute in order — semantically identical)."""
    import bass_rust
    from concourse import mybir

    n_split = 0
    for f in nc.m.functions:
        for bb in f.blocks:
            out = []
            for inst in bb.instructions:
                si = inst.sync_info
                waits = list(si.on_wait or []) if si else []
                if len(waits) > 1:
                    for i, w in enumerate(waits[:-1]):
                        nop = mybir.InstNoOp(name=f"{inst.name}-ws{i}")
                        nop.engine = inst.engine
                        nop.bass_nofuse = True
                        nop.sync_info = bass_rust.SyncInfo(
                            on_wait=[w], on_update=[]
                        )
                        out.append(nop)
                        n_split += 1
                    si.on_wait = waits[-1:]
                out.append(inst)
            bb.instructions[:] = out
    return n_split


def build_program(split_waits=True, reps=1):
    import contextlib

    import concourse.bass as bass
    import concourse.tile as tile
    from concourse import masks, mybir

    f16 = mybir.dt.float16
    f32 = mybir.dt.float32
    Alu = mybir.AluOpType
    Act = mybir.ActivationFunctionType
    Axis = mybir.AxisListType

    nc = bass.Bass()
    # host-pre-permuted layouts: partition dim second, fully contiguous DMAs
    sent = nc.declare_dram_parameter("sent", [BPC, 128, QT, D], f16, isOutput=False)
    comm = nc.declare_dram_parameter("comm", [BPC, 128, KT, D], f16, isOutput=False)
    wqt = nc.declare_dram_parameter("wqt", [128, DC, D], f16, isOutput=False)
    wk = nc.declare_dram_parameter("wk", [128, DC, D], f16, isOutput=False)
    wvt = nc.declare_dram_parameter("wvt", [128, DC, D], f16, isOutput=False)
    bq = nc.declare_dram_parameter("bq", [D], f16, isOutput=False)
    bv = nc.declare_dram_parameter("bv", [D], f16, isOutput=False)
    out = nc.declare_dram_parameter("out", [BPC, D], f32, isOutput=True)

    with tile.TileContext(nc) as tc:
      rep_loop = tc.For_i(0, reps, 1) if reps > 1 else contextlib.nullcontext()
      with rep_loop:
        with (
            tc.tile_pool(name="consts", bufs=1) as consts,
            tc.tile_pool(name="rows", bufs=1) as rows,
            tc.tile_pool(name="smalls", bufs=2) as smalls,
            tc.tile_pool(name="wp", bufs=1) as wp,
            tc.tile_pool(name="sentp", bufs=BPC) as sentp,
            tc.tile_pool(name="commp", bufs=BPC * NSLAB) as commp,
            tc.tile_pool(name="wbp", bufs=BPC) as wbp,
            tc.tile_pool(name="scr", bufs=1) as scr,
            tc.tile_pool(name="dramp", bufs=1, space="DRAM") as dramp,
            tc.tile_pool(name="ps", bufs=1, space="PSUM") as ps,
        ):
            # ---------------- constants (no DMA) ----------------
            ident = consts.tile([128, 128], f32)
            masks.make_identity(nc, ident[:])
            ones_col16 = consts.tile([128, 1], f16)
            nc.vector.memset(ones_col16[:], 1.0)
            ones_col32 = consts.tile([128, 1], f32)
            nc.vector.memset(ones_col32[:], 1.0)
            ones_row16 = consts.tile([1, 128], f16)
            nc.vector.memset(ones_row16[:], 1.0)
            ones_row32 = consts.tile([1, 128], f32)
            nc.vector.memset(ones_row32[:], 1.0)
            lq_row = consts.tile([1, 128], f16)
            nc.vector.memset(lq_row[:], float(LQ))
            dummy = consts.tile([1, 1], f32)
            nc.vector.memset(dummy[:], 0.0)
            nc.scalar.activation(dummy[:], dummy[:], Act.Exp)

            # ---------------- DMA issue order on the SP ring -------------
            # Wq/Wk (phase-0 critical) -> sentence -> comment -> Wv (end)
            wqt_sb = wp.tile([128, DC, D], f16, tag="wqt")
            nc.sync.dma_start(out=wqt_sb[:], in_=wqt[:])
            wk_sb = wp.tile([128, DC, D], f16, tag="wk")
            nc.sync.dma_start(out=wk_sb[:], in_=wk[:])

            sent_tiles = []
            for b in range(BPC):
                t = sentp.tile([128, QT, D], f16, tag="sent")
                nc.sync.dma_start(out=t[:], in_=sent[b])
                sent_tiles.append(t)

            slabs = {}
            for b in range(BPC):
                for s in range(NSLAB):
                    t = commp.tile([128, KPS, D], f16, tag="slab")
                    nc.sync.dma_start(
                        out=t[:], in_=comm[b, :, s * KPS : (s + 1) * KPS, :]
                    )
                    slabs[(b, s)] = t

            wvt_sb = wp.tile([128, DC, D], f16, tag="wvt")
            nc.sync.dma_start(out=wvt_sb[:], in_=wvt[:])

            # small loads on the ACT ring (bypass the bulk FIFO)
            bq_row = rows.tile([1, D], f16, tag="bq")
            nc.scalar.dma_start(out=bq_row[:], in_=bq[None, :])
            bv_row = rows.tile([1, D], f16, tag="bv")
            nc.scalar.dma_start(out=bv_row[:], in_=bv[None, :])

            # ---------------- phase 0: s_sum (column form), q_sum, w -----
            # ssT[:, c, b] = sum over q of sentence[b, q, c*128:(c+1)*128]
            ssT = smalls.tile([128, DC, BPC], f16, tag="ssT")
            for b in range(BPC):
                for c in range(DC):
                    pc = ps.tile([128, 1], f32, tag="C", bufs=3)
                    for t in range(QT):
                        nc.tensor.matmul(
                            pc[:],
                            sent_tiles[b][:, t, c * 128 : (c + 1) * 128],
                            ones_col16[:],
                            start=(t == 0), stop=(t == QT - 1),
                        )
                    nc.scalar.copy(ssT[:, c, b : b + 1], pc[:])

            # q_sumT chunks [128, DC, BPC]: Wq @ ssT + Lq*bq
            q_sumT = smalls.tile([128, DC, BPC], f16, tag="qsT")
            for m in range(DC):
                pq = ps.tile([128, BPC], f32, tag="C", bufs=3)
                for c in range(DC):
                    nc.tensor.matmul(
                        pq[:], wqt_sb[:, c, m * 128 : (m + 1) * 128],
                        ssT[:, c, :], start=(c == 0), stop=False,
                    )
                nc.tensor.matmul(
                    pq[:], bq_row[0:1, m * 128 : (m + 1) * 128],
                    lq_row[0:1, 0:BPC], start=False, stop=True,
                )
                nc.scalar.copy(q_sumT[:, m, :], pq[:])

            # wT chunks [128, DC, BPC]: Wk.T @ q_sumT
            wT = smalls.tile([128, DC, BPC], f32, tag="wT")
            for m in range(DC):
                pw = ps.tile([128, BPC], f32, tag="C", bufs=3)
                for c in range(DC):
                    nc.tensor.matmul(
                        pw[:], wk_sb[:, c, m * 128 : (m + 1) * 128],
                        q_sumT[:, c, :], start=(c == 0), stop=(c == DC - 1),
                    )
                nc.scalar.copy(wT[:, m, :], pw[:])

            # w rows [BPC, D] fp16 -> DRAM bounce for partition-broadcast
            w_rows = rows.tile([BPC, D], f16, tag="wrows")
            for m in range(DC):
                pr = ps.tile([128, 128], f32, tag="C", bufs=3)
                nc.tensor.transpose(pr[0:BPC, 0:128], wT[:, m, :], ident[:, 0:128])
                nc.scalar.copy(w_rows[:, m * 128 : (m + 1) * 128], pr[0:BPC, 0:128])
            w_dram = dramp.tile([BPC, D], f16)
            nc.scalar.dma_start(out=w_dram[:], in_=w_rows[:])

            # ---------------- main loop over batches ----------------
            import concourse.bass as bass_mod

            s_cols = smalls.tile([128, BPC, KT], f32, tag="scols")
            p_cols = smalls.tile([128, BPC, KT], f16, tag="pcols")
            l_row = rows.tile([1, BPC], f32, tag="lrow")
            ctx_flat = rows.tile([1, BPC, D], f32, tag="ctxflat")
            ctxT = smalls.tile([128, DC, BPC], f16, tag="ctxT")
            ttr_out = scr.tile([128, D], f16, tag="ttr")

            for b in range(BPC):
                # broadcast w[b] to all partitions (DRAM source, step-0 AP)
                wb = wbp.tile([128, D], f16, tag="wb")
                src = w_dram[b : b + 1, :]
                src_bcast = bass_mod.AP(
                    tensor=src.tensor, offset=src.offset,
                    ap=[[0, 128]] + list(src.ap[1:]),
                )
                nc.scalar.dma_start(out=wb[:], in_=src_bcast)

                # scores: fused mul+reduce on DVE, one op per k-tile
                for t in range(KT):
                    slab = slabs[(b, t // KPS)]
                    nc.vector.scalar_tensor_tensor(
                        out=ttr_out[:],
                        in0=slab[:, t % KPS, :],
                        scalar=1.0,
                        in1=wb[:],
                        op0=Alu.mult,
                        op1=Alu.mult,
                        accum_out=s_cols[:, b, t : t + 1],
                    )

                # softmax: row-max on DVE, cross-partition max via PE
                rowmax = smalls.tile([128, 1], f32, tag="rowmax")
                nc.vector.tensor_reduce(
                    out=rowmax[:], in_=s_cols[:, b, :], axis=Axis.X, op=Alu.max
                )
                prm = ps.tile([1, 128], f32, tag="C", bufs=3)
                nc.tensor.transpose(prm[:], rowmax[:], ident[:])
                rm_row = smalls.tile([1, 128], f32, tag="rmrow")
                nc.scalar.copy(rm_row[:], prm[:])
                M_sb = smalls.tile([1, 1], f32, tag="M")
                nc.vector.tensor_reduce(
                    out=M_sb[:], in_=rm_row[:], axis=Axis.X, op=Alu.max
                )
                pnm = ps.tile([128, 1], f32, tag="C", bufs=3)
                nc.tensor.matmul(pnm[:], ones_row32[:], M_sb[:])
                nm = smalls.tile([128, 1], f32, tag="nm")
                nc.scalar.activation(nm[:], pnm[:], Act.Copy, scale=-1.0)
                rowsum = smalls.tile([128, 1], f32, tag="rowsum")
                nc.scalar.activation(
                    p_cols[:, b, :], s_cols[:, b, :], Act.Exp,
                    bias=nm[:], scale=1.0, accum_out=rowsum[:],
                )
                pl = ps.tile([1, 1], f32, tag="C", bufs=3)
                nc.tensor.matmul(pl[:], rowsum[:], ones_col32[:])
                nc.scalar.copy(l_row[0:1, b : b + 1], pl[:])

                # ctx accumulation on PE: ctx = p @ comment
                ca = ps.tile([1, 512], f32, tag="A", bufs=2)
                cb = ps.tile([1, 256], f32, tag="Bk", bufs=2)
                for t in range(KT):
                    slab = slabs[(b, t // KPS)]
                    ch = slab[:, t % KPS, :]
                    pcol = p_cols[:, b, t : t + 1]
                    nc.tensor.matmul(ca[:], pcol, ch[:, 0:512],
                                     start=(t == 0), stop=(t == KT - 1))
                    nc.tensor.matmul(cb[:], pcol, ch[:, 512:768],
                                     start=(t == 0), stop=(t == KT - 1))
                nc.scalar.copy(ctx_flat[0:1, b, 0:512], ca[:])
                nc.scalar.copy(ctx_flat[0:1, b, 512:768], cb[:])

                # ctxT chunks for the final projection, inline per batch
                for c in range(DC):
                    pt = ps.tile([128, BPC], f32, tag="C", bufs=3)
                    nc.tensor.transpose(
                        pt[:, 0:1],
                        ctx_flat[0:1, b, c * 128 : (c + 1) * 128],
                        ident[0:1, 0:1],
                    )
                    nc.scalar.copy(ctxT[:, c, b : b + 1], pt[:, 0:1])

            # ---------------- final projection ----------------
            poa = ps.tile([BPC, 512], f32, tag="A", bufs=2)
            pob = ps.tile([BPC, 256], f32, tag="Bk", bufs=2)
            for c in range(DC):
                nc.tensor.matmul(poa[:], ctxT[:, c, :],
                                 wvt_sb[:, c, 0:512],
                                 start=(c == 0), stop=False)
                nc.tensor.matmul(pob[:], ctxT[:, c, :],
                                 wvt_sb[:, c, 512:768],
                                 start=(c == 0), stop=False)
            nc.tensor.matmul(poa[:], ones_row16[0:1, 0:BPC],
                             bv_row[0:1, 0:512],
                             start=False, stop=True)
            nc.tensor.matmul(pob[:], ones_row16[0:1, 0:BPC],
                             bv_row[0:1, 512:768],
                             start=False, stop=True)

            # 1/l per batch
            pli = ps.tile([BPC, 1], f32, tag="C", bufs=3)
            nc.tensor.transpose(pli[:], l_row[0:1, 0:BPC], ident[0:1, 0:1])
            li = smalls.tile([BPC, 1], f32, tag="li")
            nc.scalar.copy(li[:], pli[:])
            invl = smalls.tile([BPC, 1], f32, tag="invl")
            nc.vector.reciprocal(invl[:], li[:])

            out_sb = rows.tile([BPC, D], f32, tag="outsb")
            nc.vector.tensor_scalar(
                out=out_sb[:, 0:512], in0=poa[:], scalar1=invl[:], scalar2=None,
                op0=Alu.mult,
            )
            nc.vector.tensor_scalar(
                out=out_sb[:, 512:768], in0=pob[:], scalar1=invl[:], scalar2=None,
                op0=Alu.mult,
            )
            nc.scalar.dma_start(out=out[:], in_=out_sb[:])

    if split_waits:
        _split_multi_waits(nc)
    return nc


def _get_program():
    if "nc" not in _cache:
        _cache["nc"] = build_program()
    return _cache["nc"]


def _make_in_maps(sentence_rep, comment_rep, Wq, bq, Wk, bk, Wv, bv):
    del bk  # softmax is shift-invariant: the bk term cancels exactly
    f16 = np.float16

    def to16(x):
        return np.asarray(x, dtype=np.float32).astype(f16)

    # [B, L, D] -> [B, 128, L//128, D]: partition-major, contiguous slab DMAs
    sent = np.ascontiguousarray(
        to16(sentence_rep).reshape(B, QT, 128, D).transpose(0, 2, 1, 3)
    )
    comm = np.ascontiguousarray(
        to16(comment_rep).reshape(B, KT, 128, D).transpose(0, 2, 1, 3)
    )
    # [D, D] -> [128, DC, D] with w[p, c, e] = W[c*128+p, e]
    def wlay(w):
        return np.ascontiguousarray(
            to16(w).reshape(DC, 128, D).transpose(1, 0, 2)
        )

    wqt = wlay(np.asarray(Wq, dtype=np.float32).T)
    wk_ = wlay(np.asarray(Wk, dtype=np.float32))
    wvt = wlay(np.asarray(Wv, dtype=np.float32).T)
    bq_ = to16(bq)
    bv_ = to16(bv)
    in_maps = []
    for c in range(NCORES):
        sl = slice(c * BPC, (c + 1) * BPC)
        in_maps.append({
            "sent": sent[sl], "comm": comm[sl],
            "wqt": wqt, "wk": wk_, "wvt": wvt, "bq": bq_, "bv": bv_,
        })
    return in_maps


def run(inputs, trace=False, **kwargs):
    from concourse.bass_utils import run_bass_kernel_spmd

    nc = _get_program()
    in_maps = _make_in_maps(**inputs)
    res = run_bass_kernel_spmd(
        nc, in_maps, list(range(NCORES)), trace=trace, **kwargs
    )
    out = np.concatenate([res.results[c]["out"] for c in range(NCORES)], axis=0)
    return out.astype(np.float32), res


def kernel(**inputs) -> np.ndarray:
    out, _ = run(inputs)
    return out
